# revision 35
# baseline (speedup 1.0000x reference)
"""AGNN (linear/relu + 2x AGNNConv) on 8 Trainium2 NeuronCores.

Strategy (graph/data parallel, per the sharding hint):
  - Nodes are partitioned across the 8 cores (PPC=6272 padded slots each).
    The 2*PPC most src-referenced nodes go to cores 3-4, which lie in the
    OVERLAP of the two int16-addressable gather windows
    (A = rows [0, 5*PPC), B = rows [3*PPC, 8*PPC)), so each edge to them can
    be routed to either gather call; per-tile assignment balances the A/B
    chunk counts toward the single-class optimum (max in-degree per tile).
    Within a core, nodes are ordered by in-degree desc so 128-node dst
    tiles have uniform degrees.
  - Each conv builds per-node 256B table rows
    [h[0:126] bf16 | (h[126], h[127]) fp8 | rnrm bf16],
    AllGathers the shards so every core holds the full table in DRAM, then
    processes its dst nodes in tiles of 128 (cos = raw h dot scaled by the
    gathered rnrm_src and the local rnrm_dst):
      * dma_gather fetches source-node rows for all edge slots
        (slot (d, j) = j-th dst node's d-th in-edge, d-major so the segment
        reduction is a PSUM-accumulated identity matmul),
      * DVE computes all edge cosines of a tile in two batched ops
        (broadcast product + 3D reduce), ACT exponentiates into a
        feature-replicated alpha tile, DVE scales the h rows (packed bf16),
      * PE accumulates the scaled rows + an analytic self-loop chunk,
      * group-batched reciprocal normalizes; dead slots hit all-zero pad
        rows so only the denominator needs the live mask.
  - Host reassembles/unpermutes the 8 output shards.
"""

import sys

sys.path.insert(0, "/opt/trn_rl_repo")

import numpy as np
import ml_dtypes

# ----------------------------------------------------------------------------
# Problem dimensions (hardcoded; overridable for small-scale testing)
# ----------------------------------------------------------------------------
N_NODES = 50000
IN_CH = 256
HID = 128
NC = 8
PPC = 6272            # padded nodes per core (multiple of 128)

_BF16 = ml_dtypes.bfloat16

GROUP_CHUNKS = 32     # slot chunks per gather group
GROUP_TILES = 5       # max dst tiles per gather group
LOOKAHEAD = 5         # gather groups issued ahead of compute


def set_dims(n_nodes, ppc, in_ch=256):
    global N_NODES, PPC, IN_CH
    N_NODES, PPC, IN_CH = n_nodes, ppc, in_ch


CH = [0, 1664, 3200, 4736, 6272]   # within-core chunk row boundaries
CHT = [13, 25, 37, 49]             # boundary tiles (CH[k+1] / 128)


def _dims():
    nt = PPC // 128
    ntab = NC * PPC
    # chunk-major table layout: chunk k holds all 8 cores' within-core rows
    # [CH[k], CH[k+1]); gather window A = chunks 0-1, B = chunks 2-3
    choff = [0]
    for k in range(4):
        choff.append(choff[-1] + NC * (CH[k + 1] - CH[k]))
    split = choff[2]          # B gather window base row (= A window end)
    assert split <= 32768 and (ntab - split) <= 32768
    return nt, ntab, split, choff


def _install_axon_prof_hook():
    """Register the NTFF profile hook missing from this image's antenv."""
    import contextlib, ctypes, types

    name = "antenv.axon_hooks"
    if name in sys.modules:
        return
    hook = None
    try:
        lib = ctypes.CDLL("/opt/axon/libaxon_pjrt.so")
        if hasattr(lib, "axon_start_nrt_profile"):
            lib.axon_start_nrt_profile.argtypes = [
                ctypes.POINTER(ctypes.c_int64),
                ctypes.c_size_t,
            ]
            lib.axon_start_nrt_profile.restype = ctypes.c_int64
            lib.axon_stop_nrt_profile.argtypes = [ctypes.c_char_p]
            lib.axon_stop_nrt_profile.restype = ctypes.c_int64

            @contextlib.contextmanager
            def _hook(output_dir, device_ids):
                import jax

                jax.devices()
                if device_ids:
                    ids = (ctypes.c_int64 * len(device_ids))(*device_ids)
                    rc = lib.axon_start_nrt_profile(ids, len(device_ids))
                else:
                    rc = lib.axon_start_nrt_profile(None, 0)
                if rc != 0:
                    raise RuntimeError(f"axon_start_nrt_profile rc={rc}")
                try:
                    yield
                finally:
                    lib.axon_stop_nrt_profile(str(output_dir).encode())

            hook = _hook
    except OSError:
        pass
    mod = types.ModuleType(name)
    mod.get_axon_ntff_profile_hook = lambda: hook
    mod.set_axon_ntff_profile_hook = lambda h: None
    sys.modules[name] = mod


# ----------------------------------------------------------------------------
# Host-side graph preprocessing
# ----------------------------------------------------------------------------
class _Meta:
    pass


def _trow_of(rows, CHOFF):
    # chunk-major table row for global within-core row ids (c*PPC + i)
    c = rows // PPC
    i = rows % PPC
    k = np.digitize(i, CH[1:4])
    Lk = np.array([CH[j + 1] - CH[j] for j in range(4)])
    base = np.array(CH[:4])
    off = np.array(CHOFF[:4])
    return off[k] + c * Lk[k] + (i - base[k])


def prep_graph(edge_index):
    NT, NTAB, SPLIT, CHOFF = _dims()
    src_all = np.ascontiguousarray(edge_index[0]).astype(np.int64)
    dst_all = np.ascontiguousarray(edge_index[1]).astype(np.int64)
    nonself = src_all != dst_all
    src = src_all[nonself]
    dst = dst_all[nonself]
    # analytic self contributions: appended loop + coincidental (j, j) edges
    mcount = np.ones(N_NODES, np.float32)
    np.add.at(mcount, dst_all[~nonself], 1.0)

    # ---- stage 1: core assignment by src frequency (balances per-core
    # gather-table hit rates); core 0 reserves within-core row CH[1]-1 as
    # the window-A dead row, so cores 0-4 get at most PPC-1 hot nodes each.
    freq = np.bincount(src, minlength=N_NODES)
    rank = np.argsort(-freq, kind="stable")
    n_hot = min(5 * (PPC - 1), N_NODES)
    node_core = np.empty(N_NODES, np.int64)
    node_core[rank[:n_hot]] = np.arange(n_hot) % 5
    node_core[rank[n_hot:]] = 5 + np.arange(N_NODES - n_hot) % 3

    # ---- stage 2, pass 1: rows by total in-degree (estimate), giving a
    # provisional chunk-major trow -> window split -> per-window degrees
    d_n = np.zeros(N_NODES, np.int64)
    np.add.at(d_n, dst, 1)
    node2row = np.empty(N_NODES, np.int64)
    row2node = np.full(NTAB, -1, np.int64)

    def assign_rows(key_minor, key_major, per_chunk=False):
        node2row[:] = -1
        row2node[:] = -1
        for c in range(NC):
            nodes_c = np.where(node_core == c)[0]
            order = nodes_c[np.lexsort((key_minor[nodes_c],
                                        key_major[nodes_c]))]
            slots = np.arange(PPC)
            if c == 0:
                slots = np.concatenate([np.arange(CH[1] - 1),
                                        np.arange(CH[1], PPC)])
            assert len(order) <= len(slots), (c, len(order))
            if per_chunk:
                # keep each node inside its current chunk (so the A/B
                # window membership from pass 1 stays valid), re-sorting
                # only within the chunk for tile uniformity
                cur_rows = node2row_prev[order] - c * PPC
                kcur = np.digitize(cur_rows, CH[1:4])
                order = order[np.lexsort((key_minor[order],
                                          key_major[order], kcur))]
            node2row[order] = c * PPC + slots[: len(order)]
            row2node[c * PPC + slots[: len(order)]] = order

    assign_rows(-d_n, -d_n)
    node2row_prev = node2row.copy()
    trow1 = _trow_of(node2row[src], CHOFF)
    inA1 = trow1 < SPLIT
    dA_n = np.zeros(N_NODES, np.int64)
    dB_n = np.zeros(N_NODES, np.int64)
    np.add.at(dA_n, dst[inA1], 1)
    np.add.at(dB_n, dst[~inA1], 1)

    # ---- stage 2, pass 2: final rows by (dA, dB) desc for tile uniformity
    assign_rows(-dB_n, -dA_n, per_chunk=True)

    meta = _Meta()
    meta.node2row = node2row
    meta.row2node = row2node

    srow = _trow_of(node2row[src], CHOFF)
    drow = node2row[dst]
    e_inA = srow < SPLIT
    # dead-slot sentinels: pad rows (all-zero table entries)
    deadA = CH[1] - 1                     # core 0 chunk-0 tail (reserved)
    deadB = (NTAB - 1) - SPLIT            # core 7 tail
    assert row2node[0 * PPC + CH[1] - 1] == -1
    assert row2node[7 * PPC + PPC - 1] == -1

    # ---- stage 3: tile shapes (shared across cores for SPMD)
    a0 = np.zeros(NTAB, np.int64)
    b0 = np.zeros(NTAB, np.int64)
    np.add.at(a0, drow[e_inA], 1)
    np.add.at(b0, drow[~e_inA], 1)
    a0t = a0.reshape(NC, NT, 128)
    b0t = b0.reshape(NC, NT, 128)
    CA = a0t.max(axis=(0, 2))
    CB = b0t.max(axis=(0, 2))

    # group tiles so gather calls amortize per-call cost; within a group
    # all tiles share the group-max chunk counts so the per-group DVE ops
    # can run batched over a flat slot range
    groups = []
    cur, curC = [], 0
    for t in range(NT):
        c = int(CA[t] + CB[t])
        if cur and (curC + c > GROUP_CHUNKS or len(cur) >= GROUP_TILES):
            groups.append(cur)
            cur, curC = [], 0
        cur.append(t)
        curC += c
    if cur:
        groups.append(cur)
    meta.groups = groups
    gCA = [int(max(CA[t] for t in g)) for g in groups]
    gCB = [int(max(CB[t] for t in g)) for g in groups]
    meta.gCA = gCA
    meta.gCB = gCB
    goffs = []
    o = 0
    for gi, g in enumerate(groups):
        goffs.append(o)
        o += len(g) * (gCA[gi] + gCB[gi])
    meta.goffs = goffs
    meta.totC = o

    # ---- stage 4: per-core slot streams (per group: A-flat then B-flat,
    # tile-major, each tile padded to the group-max chunk count)
    idx16_all, live_all, mvec_all = [], [], []
    for c in range(NC):
        sel = (drow // PPC) == c
        d_local = drow[sel] % PPC
        s_r = srow[sel]
        inA = e_inA[sel]
        e_order = np.argsort(d_local, kind="stable")
        s_sorted = s_r[e_order]
        inA_s = inA[e_order]
        starts = np.zeros(PPC + 1, np.int64)
        starts[1:] = np.cumsum(np.bincount(d_local, minlength=PPC))

        idx_stream = []
        live = np.zeros((128, meta.totC), _BF16)
        mv = np.zeros((128, NT), np.float32)
        for gi, g in enumerate(groups):
            for useA in (True, False):
                cn = gCA[gi] if useA else gCB[gi]
                if cn == 0:
                    continue
                for i, t in enumerate(g):
                    idxT = np.full((cn, 128), deadA if useA else deadB,
                                   np.int64)
                    base = t * 128
                    coff = goffs[gi] + (
                        i * cn if useA
                        else len(g) * gCA[gi] + i * cn)
                    for j in range(128):
                        lp = base + j
                        st, en = starts[lp], starts[lp + 1]
                        sj = s_sorted[st:en]
                        sel_j = inA_s[st:en] if useA else ~inA_s[st:en]
                        a = sj[sel_j] if useA else sj[sel_j] - SPLIT
                        assert len(a) <= cn, (t, j, len(a), cn)
                        idxT[: len(a), j] = a
                        live[j, coff : coff + len(a)] = 1.0
                    idx_stream.append(idxT.reshape(-1))
        for t in range(NT):
            nodes = meta.row2node[c * PPC + t * 128 + np.arange(128)]
            valid = nodes >= 0
            mv[valid, t] = mcount[nodes[valid]]
        flat = (
            np.concatenate(idx_stream) if idx_stream else np.zeros(0, np.int64)
        )
        assert flat.max(initial=0) < 32768 and flat.min(initial=0) >= 0
        n = len(flat)
        a16 = np.zeros((16, max(1, (n + 15) // 16)), np.int16)
        a16[np.arange(n) % 16, np.arange(n) // 16] = flat.astype(np.int16)
        idx16_all.append(np.ascontiguousarray(np.tile(a16, (8, 1))))
        live_all.append(live)
        mvec_all.append(mv)

    meta.idx16 = idx16_all
    meta.tot16 = idx16_all[0].shape[1]
    meta.live = live_all
    meta.mvec = mvec_all
    meta.slots = int(128 * meta.totC)
    return meta


def prep_inputs(x, W1, b1, meta):
    NT, NTAB, SPLIT, CHOFF = _dims()
    xT3 = np.zeros((NC, 3, 128, PPC), _BF16)
    for c in range(NC):
        nodes = meta.row2node[c * PPC : (c + 1) * PPC]
        valid = nodes >= 0
        xa = np.zeros((PPC, IN_CH), np.float32)
        xa[valid] = x[nodes[valid]]
        xt = np.zeros((384, PPC), np.float32)
        xt[:IN_CH] = xa.T
        xt[IN_CH, :] = valid.astype(np.float32)
        xt3 = xt.astype(_BF16)
        xT3[c, 0] = xt3[:128]
        xT3[c, 1] = xt3[128:256]
        xT3[c, 2] = xt3[256:384]
    W1T3 = np.zeros((3, 128, HID), np.float32)
    W1T3[0] = W1.T[:128]
    W1T3[1] = W1.T[128:IN_CH]
    W1T3[2, 0, :] = b1
    ident = np.eye(128, dtype=_BF16)
    return xT3, W1T3.astype(_BF16), ident


# ----------------------------------------------------------------------------
# Device kernel builder
# ----------------------------------------------------------------------------
def build_nc(meta, l1_group=4):
    import concourse.bacc as bacc
    import concourse.tile as tile
    import concourse.mybir as mybir

    NT, NTAB, SPLIT, CHOFF = _dims()
    dt = mybir.dt
    AX = mybir.AxisListType
    ALU = mybir.AluOpType
    ACTF = mybir.ActivationFunctionType

    nc = bacc.Bacc("TRN2", target_bir_lowering=False, debug=False,
                   num_devices=NC, num_swdge_queues=4)

    xT3_p = nc.dram_tensor("xT3", [3, 128, PPC], dt.bfloat16, kind="ExternalInput")
    W1T3_p = nc.dram_tensor("W1T3", [3, 128, HID], dt.bfloat16, kind="ExternalInput")
    idx_p = nc.dram_tensor("idx16", [128, meta.tot16], dt.int16, kind="ExternalInput")
    live_p = nc.dram_tensor("live", [128, meta.totC], dt.bfloat16, kind="ExternalInput")
    mvec_p = nc.dram_tensor("mvec", [128, NT], dt.float32, kind="ExternalInput")
    ident_p = nc.dram_tensor("ident", [128, 128], dt.bfloat16, kind="ExternalInput")
    beta2_p = nc.dram_tensor("beta2", [1, 1], dt.float32, kind="ExternalInput")
    out_p = nc.dram_tensor("out", [PPC, HID], dt.float32, kind="ExternalOutput")

    # table row (256B): [h[0:126] bf16 | (h[126], h[127]) fp8 | rnrm bf16]
    shard1 = nc.dram_tensor("shard1", [PPC, 128], dt.bfloat16)
    shard2 = nc.dram_tensor("shard2", [PPC, 128], dt.bfloat16)
    table1 = nc.dram_tensor("table1", [NTAB, 128], dt.bfloat16, addr_space="Shared")
    table2 = nc.dram_tensor("table2", [NTAB, 128], dt.bfloat16, addr_space="Shared")
    # private per-core copies: spread the random gather reads across each
    # core's own HBM instead of all 8 cores hammering the one shared
    # buffer; split by gather window so A-gathers can start as soon as
    # chunks 0-1 have been staged
    table1pA = nc.dram_tensor("table1pA", [SPLIT, 128], dt.bfloat16)
    table1pB = nc.dram_tensor("table1pB", [NTAB - SPLIT, 128], dt.bfloat16)
    table2pA = nc.dram_tensor("table2pA", [SPLIT, 128], dt.bfloat16)
    table2pB = nc.dram_tensor("table2pB", [NTAB - SPLIT, 128], dt.bfloat16)

    rg = [list(range(NC))]
    qctr = [0]

    with tile.TileContext(nc) as tc:
        with (
            tc.tile_pool(name="persist", bufs=1) as pp,
            tc.tile_pool(name="nodes", bufs=1) as np_pool,
            tc.tile_pool(name="gath", bufs=LOOKAHEAD + 1) as gp,
            tc.tile_pool(name="work", bufs=2) as wp,
            tc.tile_pool(name="xtile", bufs=2) as xtp,
            tc.tile_pool(name="small", bufs=6) as sp,
            tc.tile_pool(name="grp", bufs=2) as grp_pool,
            tc.tile_pool(name="psum", bufs=6, space="PSUM") as psp,
        ):
            idx_sb = pp.tile([128, meta.tot16], dt.int16)
            live_sb = pp.tile([128, meta.totC], dt.bfloat16)
            mvec_sb = pp.tile([128, NT], dt.float32)
            ident_sb = pp.tile([128, 128], dt.bfloat16)
            w1_sb = pp.tile([128, 3, HID], dt.bfloat16)
            beta2_sb = pp.tile([1, 1], dt.float32)
            beta2_col = pp.tile([128, 1], dt.float32)
            nc.sync.dma_start(idx_sb[:], idx_p[:])
            nc.sync.dma_start(live_sb[:], live_p[:])
            nc.sync.dma_start(mvec_sb[:], mvec_p[:])
            nc.sync.dma_start(ident_sb[:], ident_p[:])
            for k in range(3):
                nc.sync.dma_start(w1_sb[:, k, :], W1T3_p[k])
            nc.sync.dma_start(beta2_sb[:], beta2_p[:])
            nc.gpsimd.partition_broadcast(beta2_col[:], beta2_sb[:])

            h_nm = np_pool.tile([128, NT, 128], dt.bfloat16)
            rnrm_nm = np_pool.tile([128, NT], dt.float32)

            def rsqrt_batch(nrm2, G):
                # rnrm = rsqrt(max(nrm2, 1e-24)) via bit magic + 2 Newton
                # steps; nrm2 is a [128, G] f32 tile, modified in place.
                nc.vector.tensor_scalar_max(out=nrm2[:], in0=nrm2[:],
                                            scalar1=1e-24)
                rnrm = sp.tile([128, G], dt.float32, tag="rnrm")
                bits = rnrm[:].bitcast(dt.int32)
                nc.vector.tensor_scalar(
                    out=bits, in0=nrm2[:].bitcast(dt.int32), scalar1=1,
                    scalar2=None, op0=ALU.logical_shift_right,
                )
                nc.vector.tensor_scalar(
                    out=bits, in0=bits, scalar1=-1, scalar2=0x5F3759DF,
                    op0=ALU.mult, op1=ALU.add,
                )
                tmp = sp.tile([128, G], dt.float32, tag="rsq_t")
                for _ in range(2):
                    nc.vector.tensor_mul(out=tmp[:], in0=rnrm[:], in1=rnrm[:])
                    nc.vector.tensor_mul(out=tmp[:], in0=tmp[:], in1=nrm2[:])
                    nc.vector.tensor_scalar(
                        out=tmp[:], in0=tmp[:], scalar1=-0.5, scalar2=1.5,
                        op0=ALU.mult, op1=ALU.add,
                    )
                    nc.vector.tensor_mul(out=rnrm[:], in0=rnrm[:], in1=tmp[:])
                return rnrm

            def norm_and_store(ts, shard):
                # h_nm[:, t, :] already holds h (bf16) for t in ts; compute
                # rnrm, build the packed 256B rows, and write the shard.
                G = len(ts)
                t0, G0 = ts[0], len(ts)
                nrm2 = sp.tile([128, G], dt.float32, tag="nrm2")
                sq = wp.tile([128, G, 128], dt.bfloat16, tag="sqscratch")
                nc.vector.tensor_mul(out=sq[:], in0=h_nm[:, t0 : t0 + G, :],
                                     in1=h_nm[:, t0 : t0 + G, :])
                nc.vector.tensor_reduce(
                    out=nrm2[:], in_=sq[:], axis=AX.X, op=ALU.add,
                )
                rnrm = rsqrt_batch(nrm2, G)
                nc.vector.tensor_scalar(
                    out=rnrm_nm[:, t0 : t0 + G0], in0=rnrm[:],
                    scalar1=1.0, scalar2=None, op0=ALU.mult,
                )
                row = wp.tile([128, G, 128], dt.bfloat16, tag="rowt")
                nc.vector.tensor_scalar(
                    out=row[:, :, 0:126], in0=h_nm[:, t0 : t0 + G0, 0:126],
                    scalar1=1.0, scalar2=None, op0=ALU.mult,
                )
                nc.vector.tensor_scalar(
                    out=row[:, :, 126:127].bitcast(dt.float8e4),
                    in0=h_nm[:, t0 : t0 + G0, 126:128],
                    scalar1=1.0, scalar2=None, op0=ALU.mult,
                )
                nc.vector.tensor_scalar(
                    out=row[:, :, 127:128], in0=rnrm[:].unsqueeze(2),
                    scalar1=1.0, scalar2=None, op0=ALU.mult,
                )
                nc.sync.dma_start(
                    shard[t0 * 128 : (t0 + G0) * 128, 0:128]
                    .rearrange("(g p) f -> p g f", p=128),
                    row[:],
                )

            # chunked AllGather + private-copy staging: chunk k ships as
            # soon as the tiles covering its rows are written, overlapping
            # the collective and table copy with compute
            def make_stager(shard, table, tablepA, tablepB):
                state = [0]

                def stage(tiles_done):
                    while state[0] < 4 and tiles_done >= CHT[state[0]]:
                        k = state[0]
                        r0, r1 = CH[k], CH[k + 1]
                        o0, o1 = CHOFF[k], CHOFF[k + 1]
                        nc.gpsimd.collective_compute(
                            "AllGather", mybir.AluOpType.bypass,
                            ins=[shard[r0:r1, :]], outs=[table[o0:o1, :]],
                            replica_groups=rg,
                        )
                        if k < 2:
                            dst = tablepA[o0:o1, :]
                        else:
                            dst = tablepB[o0 - SPLIT : o1 - SPLIT, :]
                        nc.sync.dma_start(dst, table[o0:o1, :])
                        state[0] += 1
                return stage

            stage1 = make_stager(shard1, table1, table1pA, table1pB)
            stage2 = make_stager(shard2, table2, table2pA, table2pB)

            # ---------------- phase 1: L1 + table1 ----------------
            def build_table1():
                for t0 in range(0, NT, l1_group):
                    ts = list(range(t0, min(t0 + l1_group, NT)))
                    G0 = len(ts)
                    xt_g = xtp.tile([128, 3, l1_group * 128], dt.bfloat16,
                                    tag="xt")
                    nc.sync.dma_start(
                        xt_g[:, :, 0 : G0 * 128],
                        xT3_p[:, :, t0 * 128 : (t0 + G0) * 128]
                        .transpose([1, 0, 2]),
                    )
                    for i, t in enumerate(ts):
                        ps = psp.tile([128, HID], dt.float32, tag="acc")
                        for k in range(3):
                            nc.tensor.matmul(
                                ps[:],
                                lhsT=xt_g[:, k, i * 128 : (i + 1) * 128],
                                rhs=w1_sb[:, k, :],
                                start=(k == 0),
                                stop=(k == 2),
                            )
                        nc.scalar.activation(h_nm[:, t, :], ps[:], ACTF.Relu)
                    norm_and_store(ts, shard1)
                    stage1(ts[-1] + 1)

            build_table1()

            # ---------------- conv phases ----------------
            def run_conv(tableA, tableB, beta_scale, beta_col_ap, writer):
                expb = sp.tile([128, 1], dt.float32, tag="expb")
                if beta_col_ap is None:
                    ones = sp.tile([128, 1], dt.float32, tag="ones1")
                    nc.vector.memset(ones[:], 1.0)
                    nc.scalar.activation(expb[:], ones[:], ACTF.Exp)
                else:
                    nc.scalar.activation(expb[:], beta_col_ap[:], ACTF.Exp)
                tabA = tableA[:]
                tabB = tableB[:]

                groups = meta.groups
                ngrp = len(groups)
                # per-group gather layout (tiles padded to group-max counts)
                gsumA = [len(g) * meta.gCA[gi] for gi, g in enumerate(groups)]
                gsumB = [len(g) * meta.gCB[gi] for gi, g in enumerate(groups)]
                gtiles = {}
                i16offs = []
                i16off = 0
                for gi, g in enumerate(groups):
                    i16offs.append(i16off)
                    i16off += (gsumA[gi] + gsumB[gi]) * 8

                def issue_gather(gi):
                    g = groups[gi]
                    sumA, sumB = gsumA[gi], gsumB[gi]
                    gtA = gp.tile([128, max(sumA, 1), 128], dt.bfloat16,
                                  tag="gA")
                    gtB = gp.tile([128, max(sumB, 1), 128], dt.bfloat16,
                                  tag="gB")
                    gtiles[gi] = (gtA, gtB)
                    off16 = i16offs[gi]
                    for (cn, gt, tab) in ((sumA, gtA, tabA),
                                          (sumB, gtB, tabB)):
                        if cn == 0:
                            continue
                        # split each call across queues so the SWDGE drains
                        # of one group run in parallel
                        half = (cn + 1) // 2
                        for (c0, c1) in ((0, half), (half, cn)):
                            if c1 <= c0:
                                continue
                            nidx = (c1 - c0) * 128
                            n16 = nidx // 16
                            nc.gpsimd.dma_gather(
                                out_ap=gt[:, c0:c1, :],
                                in_ap=tab,
                                idxs_ap=idx_sb[:, off16 : off16 + n16],
                                num_idxs=nidx,
                                num_idxs_reg=nidx,
                                elem_size=128,
                                single_packet=False,
                                queue_num=qctr[0] % 4,
                            )
                            qctr[0] += 1
                            off16 += n16

                def conv_group(gi):
                    g = groups[gi]
                    gtA, gtB = gtiles.pop(gi)
                    G = len(g)
                    CAg, CBg = meta.gCA[gi], meta.gCB[gi]
                    SA, SB = G * CAg, G * CBg
                    S = SA + SB
                    goff = meta.goffs[gi]
                    t0 = g[0]

                    # self-loop scale per tile (batched over the group)
                    selfa = sp.tile([128, G], dt.float32, tag="selfa")
                    nc.vector.tensor_scalar(
                        out=selfa[:], in0=mvec_sb[:, t0 : t0 + G],
                        scalar1=expb[:], scalar2=None, op0=ALU.mult,
                    )
                    deng = sp.tile([128, G], dt.float32, tag="deng")

                    if S > 0:
                        prod = wp.tile([128, S, 128], dt.bfloat16, tag="prod")
                        cosg = sp.tile([128, S], dt.float32, tag="cosg")
                        for (cn, SN, po, gt) in ((CAg, SA, 0, gtA),
                                                 (CBg, SB, SA, gtB)):
                            if SN == 0:
                                continue
                            g4 = gt[:, 0:SN, 0:126].rearrange(
                                "p (g c) f -> p g c f", g=G)
                            nc.vector.tensor_tensor(
                                out=prod[:, po : po + SN, 0:126].rearrange(
                                    "p (g c) f -> p g c f", g=G),
                                in0=g4,
                                in1=h_nm[:, t0 : t0 + G, 0:126].unsqueeze(2)
                                .broadcast_to([128, G, cn, 126]),
                                op=ALU.mult,
                            )
                            g8 = gt[:, 0:SN, 126:127].bitcast(
                                dt.float8e4).rearrange(
                                "p (g c) f -> p g c f", g=G)
                            nc.vector.tensor_tensor(
                                out=prod[:, po : po + SN, 126:128].rearrange(
                                    "p (g c) f -> p g c f", g=G),
                                in0=g8,
                                in1=h_nm[:, t0 : t0 + G, 126:128].unsqueeze(2)
                                .broadcast_to([128, G, cn, 2]),
                                op=ALU.mult,
                            )
                            nc.vector.tensor_reduce(
                                out=cosg[:, po : po + SN],
                                in_=prod[:, po : po + SN, :],
                                axis=AX.X, op=ALU.add,
                            )
                            # raw dot -> cosine: * rnrm_src (gathered col
                            # 127) and * rnrm_dst (local, per tile)
                            nc.vector.tensor_tensor(
                                out=cosg[:, po : po + SN].unsqueeze(2),
                                in0=cosg[:, po : po + SN].unsqueeze(2),
                                in1=gt[:, 0:SN, 127:128],
                                op=ALU.mult,
                            )
                            nc.vector.tensor_tensor(
                                out=cosg[:, po : po + SN].rearrange(
                                    "p (g c) -> p g c", g=G),
                                in0=cosg[:, po : po + SN].rearrange(
                                    "p (g c) -> p g c", g=G),
                                in1=rnrm_nm[:, t0 : t0 + G].unsqueeze(2)
                                .broadcast_to([128, G, cn]),
                                op=ALU.mult,
                            )
                        alpha = sp.tile([128, S], dt.bfloat16, tag="alpha")
                        scale = beta_scale if beta_col_ap is None \
                            else beta_col_ap[:]
                        nc.scalar.activation(alpha[:], cosg[:], ACTF.Exp,
                                             scale=scale)
                        # denominator: alpha * live, reduced per tile
                        alpham = sp.tile([128, S], dt.float32, tag="alpham")
                        nc.vector.tensor_tensor(
                            out=alpham[:], in0=alpha[:],
                            in1=live_sb[:, goff : goff + S], op=ALU.mult,
                        )
                        den0 = sp.tile([128, 2 * G], dt.float32, tag="den0")
                        for (SN, po, do) in ((SA, 0, 0), (SB, SA, G)):
                            if SN:
                                nc.vector.tensor_reduce(
                                    out=den0[:, do : do + G],
                                    in_=alpham[:, po : po + SN].rearrange(
                                        "p (g c) -> p g c", g=G),
                                    axis=AX.X, op=ALU.add,
                                )
                            else:
                                nc.vector.memset(den0[:, do : do + G], 0.0)
                        dsum = sp.tile([128, G], dt.float32, tag="dsum")
                        nc.vector.tensor_tensor(
                            out=dsum[:], in0=den0[:, 0:G],
                            in1=den0[:, G : 2 * G], op=ALU.add,
                        )
                        nc.vector.scalar_tensor_tensor(
                            out=deng[:], in0=dsum[:], scalar=1e-16,
                            in1=selfa[:], op0=ALU.add, op1=ALU.add,
                        )
                        # scaled h rows (alpha broadcast along features)
                        for (cn, SN, po, gt) in ((CAg, SA, 0, gtA),
                                                 (CBg, SB, SA, gtB)):
                            if SN == 0:
                                continue
                            nc.vector.tensor_tensor(
                                out=prod[:, po : po + SN, 0:126],
                                in0=gt[:, 0:SN, 0:126],
                                in1=alpha[:, po : po + SN].unsqueeze(2)
                                .broadcast_to([128, SN, 126]),
                                op=ALU.mult,
                            )
                            nc.vector.tensor_tensor(
                                out=prod[:, po : po + SN, 126:128],
                                in0=gt[:, 0:SN, 126:127].bitcast(
                                    dt.float8e4),
                                in1=alpha[:, po : po + SN].unsqueeze(2)
                                .broadcast_to([128, SN, 2]),
                                op=ALU.mult,
                            )
                    else:
                        nc.vector.tensor_scalar(
                            out=deng[:], in0=selfa[:],
                            scalar1=1e-16, scalar2=None, op0=ALU.add,
                        )
                    # self chunks + per-tile matmul accumulation; drain each
                    # PSUM (unscaled) on ACT so nothing here waits on den
                    pself = wp.tile([128, G, 128], dt.bfloat16, tag="pself")
                    num = wp.tile([128, G, 128], dt.float32, tag="num")
                    nc.vector.tensor_tensor(
                        out=pself[:], in0=h_nm[:, t0 : t0 + G, :],
                        in1=selfa[:].unsqueeze(2).broadcast_to([128, G, 128]),
                        op=ALU.mult,
                    )
                    for i, t in enumerate(g):
                        ps = psp.tile([128, 128], dt.float32, tag="acc")
                        C = CAg + CBg
                        for cc in range(CAg):
                            nc.tensor.matmul(
                                ps[:], lhsT=ident_sb[:],
                                rhs=prod[:, i * CAg + cc, :],
                                start=(cc == 0), stop=False,
                            )
                        for cc in range(CBg):
                            nc.tensor.matmul(
                                ps[:], lhsT=ident_sb[:],
                                rhs=prod[:, SA + i * CBg + cc, :],
                                start=False, stop=False,
                            )
                        nc.tensor.matmul(
                            ps[:], lhsT=ident_sb[:], rhs=pself[:, i, :],
                            start=(C == 0), stop=True,
                        )
                        nc.scalar.activation(num[:, i, :], ps[:], ACTF.Copy)
                    state[gi] = (g, deng, num)

                def conv_phase2(gi):
                    g, deng, num = state.pop(gi)
                    G = len(g)
                    rden = sp.tile([128, G], dt.float32, tag="rdeng")
                    nc.vector.reciprocal(rden[:], deng[:])
                    writer(g, num, rden)

                state = {}
                for gi in range(ngrp + LOOKAHEAD + 1):
                    if gi < ngrp:
                        issue_gather(gi)
                    if LOOKAHEAD <= gi < ngrp + LOOKAHEAD:
                        conv_group(gi - LOOKAHEAD)
                    if gi > LOOKAHEAD:
                        conv_phase2(gi - LOOKAHEAD - 1)

            def conv1_writer(g, num, rden):
                G = len(g)
                nc.vector.tensor_tensor(
                    out=h_nm[:, g[0] : g[0] + G, :], in0=num[:],
                    in1=rden[:].unsqueeze(2).broadcast_to([128, G, 128]),
                    op=ALU.mult,
                )
                norm_and_store(list(g), shard2)
                stage2(g[-1] + 1)

            run_conv(table1pA, table1pB, 1.0, None, conv1_writer)

            def conv2_writer(g, num, rden):
                G = len(g)
                outg = grp_pool.tile([128, G, 128], dt.float32, tag="outg")
                nc.vector.tensor_tensor(
                    out=outg[:], in0=num[:],
                    in1=rden[:].unsqueeze(2).broadcast_to([128, G, 128]),
                    op=ALU.mult,
                )
                t0 = g[0]
                nc.sync.dma_start(
                    out_p[t0 * 128 : (t0 + G) * 128, :]
                    .rearrange("(g p) f -> p g f", p=128),
                    outg[:],
                )

            run_conv(table2pA, table2pB, None, beta2_col, conv2_writer)

    nc.compile()
    return nc


# ----------------------------------------------------------------------------
# Entry point
# ----------------------------------------------------------------------------
_CACHE = {}


def make_in_maps(x, W1, b1, beta2, meta):
    xT3, W1T3, ident = prep_inputs(x, W1, b1, meta)
    in_maps = []
    for c in range(NC):
        in_maps.append({
            "xT3": np.ascontiguousarray(xT3[c]),
            "W1T3": W1T3,
            "idx16": meta.idx16[c],
            "live": meta.live[c],
            "mvec": meta.mvec[c],
            "ident": ident,
            "beta2": np.asarray(beta2, np.float32).reshape(1, 1),
        })
    return in_maps


def assemble_out(results, meta):
    out = np.zeros((N_NODES, HID), np.float32)
    for c in range(NC):
        shard = results[c]["out"]
        nodes = meta.row2node[c * PPC : (c + 1) * PPC]
        valid = nodes >= 0
        out[nodes[valid]] = shard[valid]
    return out


def kernel(x, edge_index, W1, b1, beta2):
    _install_axon_prof_hook()
    from concourse.bass_utils import run_bass_kernel_spmd

    x = np.asarray(x, np.float32)
    edge_index = np.asarray(edge_index)
    W1 = np.asarray(W1, np.float32)
    b1 = np.asarray(b1, np.float32)
    beta2 = np.asarray(beta2, np.float32)

    key = (x.shape, edge_index.shape)
    if key not in _CACHE:
        meta = prep_graph(edge_index)
        nc = build_nc(meta)
        _CACHE[key] = (meta, nc, hash(edge_index.tobytes()))
    meta, nc, ehash = _CACHE[key]
    if ehash != hash(edge_index.tobytes()):
        meta = prep_graph(edge_index)
        nc = build_nc(meta)
        _CACHE[key] = (meta, nc, hash(edge_index.tobytes()))
        meta, nc, ehash = _CACHE[key]

    in_maps = make_in_maps(x, W1, b1, beta2, meta)
    res = run_bass_kernel_spmd(nc, in_maps, list(range(NC)))
    return assemble_out(res.results, meta)



# revision 37
# speedup vs baseline: 1.0289x; 1.0289x over previous
"""AGNN (linear/relu + 2x AGNNConv) on 8 Trainium2 NeuronCores.

Strategy (graph/data parallel, per the sharding hint):
  - Nodes are partitioned across the 8 cores (PPC=6272 padded slots each,
    cores balanced by src frequency); within a core, nodes are ordered by
    per-window in-degree so 128-node dst tiles have uniform degrees.
  - Each conv builds per-node 256B table rows
    [h[0:126] bf16 | (h[126], h[127]) fp8 | rnrm bf16]  (halving the
    AllGather and per-edge gather traffic vs storing xn and h),
    then AllGathers the shards in 4 chunk-major chunks (chunk k = all 8
    cores' within-core rows [CH[k], CH[k+1])), each issued as soon as its
    tiles are computed so the collective overlaps compute. Each chunk is
    also copied into a PRIVATE per-core DRAM table so the random per-edge
    gather reads spread across all HBM stacks instead of saturating the
    one shared buffer (this alone was worth ~25%).
  - Per dst-node tile of 128 (slot (d, j) = j-th dst's d-th in-edge,
    d-major; two int16 gather windows A = chunks 0-1, B = chunks 2-3):
      * SWDGE dma_gather fetches 256B source rows for all edge slots,
      * DVE computes raw h dots via a broadcast product + 3D reduce, then
        scales by the gathered rnrm_src and the local rnrm_dst to get the
        cosine; ACT exponentiates the small [128, chunks] tile only,
      * DVE scales the gathered h rows by alpha (broadcast along
        features); PE accumulates them + an analytic self-loop chunk via
        identity matmuls in PSUM (the segment sum),
      * group-batched reciprocal normalizes; dead slots hit all-zero pad
        rows so only the denominator needs the live mask.
  - Host reassembles/unpermutes the 8 output shards.
"""

import sys

sys.path.insert(0, "/opt/trn_rl_repo")

import numpy as np
import ml_dtypes

# ----------------------------------------------------------------------------
# Problem dimensions (hardcoded; overridable for small-scale testing)
# ----------------------------------------------------------------------------
N_NODES = 50000
IN_CH = 256
HID = 128
NC = 8
PPC = 6272            # padded nodes per core (multiple of 128)

_BF16 = ml_dtypes.bfloat16

GROUP_CHUNKS = 32     # slot chunks per gather group
GROUP_TILES = 5       # max dst tiles per gather group
LOOKAHEAD = 4         # gather groups issued ahead of compute


def set_dims(n_nodes, ppc, in_ch=256):
    global N_NODES, PPC, IN_CH
    N_NODES, PPC, IN_CH = n_nodes, ppc, in_ch


CH = [0, 1664, 3200, 4736, 6272]   # within-core chunk row boundaries
CHT = [13, 25, 37, 49]             # boundary tiles (CH[k+1] / 128)


def _dims():
    nt = PPC // 128
    ntab = NC * PPC
    # chunk-major table layout: chunk k holds all 8 cores' within-core rows
    # [CH[k], CH[k+1]); gather window A = chunks 0-1, B = chunks 2-3
    choff = [0]
    for k in range(4):
        choff.append(choff[-1] + NC * (CH[k + 1] - CH[k]))
    split = choff[2]          # B gather window base row (= A window end)
    assert split <= 32768 and (ntab - split) <= 32768
    return nt, ntab, split, choff


def _install_axon_prof_hook():
    """Register the NTFF profile hook missing from this image's antenv."""
    import contextlib, ctypes, types

    name = "antenv.axon_hooks"
    if name in sys.modules:
        return
    hook = None
    try:
        lib = ctypes.CDLL("/opt/axon/libaxon_pjrt.so")
        if hasattr(lib, "axon_start_nrt_profile"):
            lib.axon_start_nrt_profile.argtypes = [
                ctypes.POINTER(ctypes.c_int64),
                ctypes.c_size_t,
            ]
            lib.axon_start_nrt_profile.restype = ctypes.c_int64
            lib.axon_stop_nrt_profile.argtypes = [ctypes.c_char_p]
            lib.axon_stop_nrt_profile.restype = ctypes.c_int64

            @contextlib.contextmanager
            def _hook(output_dir, device_ids):
                import jax

                jax.devices()
                if device_ids:
                    ids = (ctypes.c_int64 * len(device_ids))(*device_ids)
                    rc = lib.axon_start_nrt_profile(ids, len(device_ids))
                else:
                    rc = lib.axon_start_nrt_profile(None, 0)
                if rc != 0:
                    raise RuntimeError(f"axon_start_nrt_profile rc={rc}")
                try:
                    yield
                finally:
                    lib.axon_stop_nrt_profile(str(output_dir).encode())

            hook = _hook
    except OSError:
        pass
    mod = types.ModuleType(name)
    mod.get_axon_ntff_profile_hook = lambda: hook
    mod.set_axon_ntff_profile_hook = lambda h: None
    sys.modules[name] = mod


# ----------------------------------------------------------------------------
# Host-side graph preprocessing
# ----------------------------------------------------------------------------
class _Meta:
    pass


def _trow_of(rows, CHOFF):
    # chunk-major table row for global within-core row ids (c*PPC + i)
    c = rows // PPC
    i = rows % PPC
    k = np.digitize(i, CH[1:4])
    Lk = np.array([CH[j + 1] - CH[j] for j in range(4)])
    base = np.array(CH[:4])
    off = np.array(CHOFF[:4])
    return off[k] + c * Lk[k] + (i - base[k])


def prep_graph(edge_index):
    NT, NTAB, SPLIT, CHOFF = _dims()
    src_all = np.ascontiguousarray(edge_index[0]).astype(np.int64)
    dst_all = np.ascontiguousarray(edge_index[1]).astype(np.int64)
    nonself = src_all != dst_all
    src = src_all[nonself]
    dst = dst_all[nonself]
    # analytic self contributions: appended loop + coincidental (j, j) edges
    mcount = np.ones(N_NODES, np.float32)
    np.add.at(mcount, dst_all[~nonself], 1.0)

    # ---- stage 1: core assignment by src frequency (balances per-core
    # gather-table hit rates); core 0 reserves within-core row CH[1]-1 as
    # the window-A dead row, so cores 0-4 get at most PPC-1 hot nodes each.
    freq = np.bincount(src, minlength=N_NODES)
    rank = np.argsort(-freq, kind="stable")
    n_hot = min(5 * (PPC - 1), N_NODES)
    node_core = np.empty(N_NODES, np.int64)
    node_core[rank[:n_hot]] = np.arange(n_hot) % 5
    node_core[rank[n_hot:]] = 5 + np.arange(N_NODES - n_hot) % 3

    # ---- stage 2, pass 1: rows by total in-degree (estimate), giving a
    # provisional chunk-major trow -> window split -> per-window degrees
    d_n = np.zeros(N_NODES, np.int64)
    np.add.at(d_n, dst, 1)
    node2row = np.empty(N_NODES, np.int64)
    row2node = np.full(NTAB, -1, np.int64)

    def assign_rows(key_minor, key_major, per_chunk=False):
        node2row[:] = -1
        row2node[:] = -1
        for c in range(NC):
            nodes_c = np.where(node_core == c)[0]
            order = nodes_c[np.lexsort((key_minor[nodes_c],
                                        key_major[nodes_c]))]
            slots = np.arange(PPC)
            if c == 0:
                slots = np.concatenate([np.arange(CH[1] - 1),
                                        np.arange(CH[1], PPC)])
            assert len(order) <= len(slots), (c, len(order))
            if per_chunk:
                # keep each node inside its current chunk (so the A/B
                # window membership from pass 1 stays valid), re-sorting
                # only within the chunk for tile uniformity
                cur_rows = node2row_prev[order] - c * PPC
                kcur = np.digitize(cur_rows, CH[1:4])
                order = order[np.lexsort((key_minor[order],
                                          key_major[order], kcur))]
            node2row[order] = c * PPC + slots[: len(order)]
            row2node[c * PPC + slots[: len(order)]] = order

    assign_rows(-d_n, -d_n)
    node2row_prev = node2row.copy()
    trow1 = _trow_of(node2row[src], CHOFF)
    inA1 = trow1 < SPLIT
    dA_n = np.zeros(N_NODES, np.int64)
    dB_n = np.zeros(N_NODES, np.int64)
    np.add.at(dA_n, dst[inA1], 1)
    np.add.at(dB_n, dst[~inA1], 1)

    # ---- stage 2, pass 2: final rows by (dA, dB) desc for tile uniformity
    assign_rows(-dB_n, -dA_n, per_chunk=True)

    meta = _Meta()
    meta.node2row = node2row
    meta.row2node = row2node

    srow = _trow_of(node2row[src], CHOFF)
    drow = node2row[dst]
    e_inA = srow < SPLIT
    # dead-slot sentinels: pad rows (all-zero table entries)
    deadA = CH[1] - 1                     # core 0 chunk-0 tail (reserved)
    deadB = (NTAB - 1) - SPLIT            # core 7 tail
    assert row2node[0 * PPC + CH[1] - 1] == -1
    assert row2node[7 * PPC + PPC - 1] == -1

    # ---- stage 3: tile shapes (shared across cores for SPMD)
    a0 = np.zeros(NTAB, np.int64)
    b0 = np.zeros(NTAB, np.int64)
    np.add.at(a0, drow[e_inA], 1)
    np.add.at(b0, drow[~e_inA], 1)
    a0t = a0.reshape(NC, NT, 128)
    b0t = b0.reshape(NC, NT, 128)
    CA = a0t.max(axis=(0, 2))
    CB = b0t.max(axis=(0, 2))

    # group tiles so gather calls amortize per-call cost; within a group
    # all tiles share the group-max chunk counts so the per-group DVE ops
    # can run batched over a flat slot range
    groups = []
    cur, curC = [], 0
    for t in range(NT):
        c = int(CA[t] + CB[t])
        if cur and (curC + c > GROUP_CHUNKS or len(cur) >= GROUP_TILES):
            groups.append(cur)
            cur, curC = [], 0
        cur.append(t)
        curC += c
    if cur:
        groups.append(cur)
    meta.groups = groups
    gCA = [int(max(CA[t] for t in g)) for g in groups]
    gCB = [int(max(CB[t] for t in g)) for g in groups]
    meta.gCA = gCA
    meta.gCB = gCB
    goffs = []
    o = 0
    for gi, g in enumerate(groups):
        goffs.append(o)
        o += len(g) * (gCA[gi] + gCB[gi])
    meta.goffs = goffs
    meta.totC = o

    # ---- stage 4: per-core slot streams (per group: A-flat then B-flat,
    # tile-major, each tile padded to the group-max chunk count)
    idx16_all, live_all, mvec_all = [], [], []
    for c in range(NC):
        sel = (drow // PPC) == c
        d_local = drow[sel] % PPC
        s_r = srow[sel]
        inA = e_inA[sel]
        e_order = np.argsort(d_local, kind="stable")
        s_sorted = s_r[e_order]
        inA_s = inA[e_order]
        starts = np.zeros(PPC + 1, np.int64)
        starts[1:] = np.cumsum(np.bincount(d_local, minlength=PPC))

        idx_stream = []
        live = np.zeros((128, meta.totC), _BF16)
        mv = np.zeros((128, NT), np.float32)
        for gi, g in enumerate(groups):
            for useA in (True, False):
                cn = gCA[gi] if useA else gCB[gi]
                if cn == 0:
                    continue
                for i, t in enumerate(g):
                    idxT = np.full((cn, 128), deadA if useA else deadB,
                                   np.int64)
                    base = t * 128
                    coff = goffs[gi] + (
                        i * cn if useA
                        else len(g) * gCA[gi] + i * cn)
                    for j in range(128):
                        lp = base + j
                        st, en = starts[lp], starts[lp + 1]
                        sj = s_sorted[st:en]
                        sel_j = inA_s[st:en] if useA else ~inA_s[st:en]
                        a = sj[sel_j] if useA else sj[sel_j] - SPLIT
                        assert len(a) <= cn, (t, j, len(a), cn)
                        idxT[: len(a), j] = a
                        live[j, coff : coff + len(a)] = 1.0
                    idx_stream.append(idxT.reshape(-1))
        for t in range(NT):
            nodes = meta.row2node[c * PPC + t * 128 + np.arange(128)]
            valid = nodes >= 0
            mv[valid, t] = mcount[nodes[valid]]
        flat = (
            np.concatenate(idx_stream) if idx_stream else np.zeros(0, np.int64)
        )
        assert flat.max(initial=0) < 32768 and flat.min(initial=0) >= 0
        n = len(flat)
        a16 = np.zeros((16, max(1, (n + 15) // 16)), np.int16)
        a16[np.arange(n) % 16, np.arange(n) // 16] = flat.astype(np.int16)
        idx16_all.append(np.ascontiguousarray(np.tile(a16, (8, 1))))
        live_all.append(live)
        mvec_all.append(mv)

    meta.idx16 = idx16_all
    meta.tot16 = idx16_all[0].shape[1]
    meta.live = live_all
    meta.mvec = mvec_all
    meta.slots = int(128 * meta.totC)
    return meta


def prep_inputs(x, W1, b1, meta):
    NT, NTAB, SPLIT, CHOFF = _dims()
    xT3 = np.zeros((NC, 3, 128, PPC), _BF16)
    for c in range(NC):
        nodes = meta.row2node[c * PPC : (c + 1) * PPC]
        valid = nodes >= 0
        xa = np.zeros((PPC, IN_CH), np.float32)
        xa[valid] = x[nodes[valid]]
        xt = np.zeros((384, PPC), np.float32)
        xt[:IN_CH] = xa.T
        xt[IN_CH, :] = valid.astype(np.float32)
        xt3 = xt.astype(_BF16)
        xT3[c, 0] = xt3[:128]
        xT3[c, 1] = xt3[128:256]
        xT3[c, 2] = xt3[256:384]
    W1T3 = np.zeros((3, 128, HID), np.float32)
    W1T3[0] = W1.T[:128]
    W1T3[1] = W1.T[128:IN_CH]
    W1T3[2, 0, :] = b1
    ident = np.eye(128, dtype=_BF16)
    return xT3, W1T3.astype(_BF16), ident


# ----------------------------------------------------------------------------
# Device kernel builder
# ----------------------------------------------------------------------------
def build_nc(meta, l1_group=4):
    import concourse.bacc as bacc
    import concourse.tile as tile
    import concourse.mybir as mybir

    NT, NTAB, SPLIT, CHOFF = _dims()
    dt = mybir.dt
    AX = mybir.AxisListType
    ALU = mybir.AluOpType
    ACTF = mybir.ActivationFunctionType

    nc = bacc.Bacc("TRN2", target_bir_lowering=False, debug=False,
                   num_devices=NC, num_swdge_queues=4)

    xT3_p = nc.dram_tensor("xT3", [3, 128, PPC], dt.bfloat16, kind="ExternalInput")
    W1T3_p = nc.dram_tensor("W1T3", [3, 128, HID], dt.bfloat16, kind="ExternalInput")
    idx_p = nc.dram_tensor("idx16", [128, meta.tot16], dt.int16, kind="ExternalInput")
    live_p = nc.dram_tensor("live", [128, meta.totC], dt.bfloat16, kind="ExternalInput")
    mvec_p = nc.dram_tensor("mvec", [128, NT], dt.float32, kind="ExternalInput")
    ident_p = nc.dram_tensor("ident", [128, 128], dt.bfloat16, kind="ExternalInput")
    beta2_p = nc.dram_tensor("beta2", [1, 1], dt.float32, kind="ExternalInput")
    out_p = nc.dram_tensor("out", [PPC, HID], dt.float32, kind="ExternalOutput")

    # table row (256B): [h[0:126] bf16 | (h[126], h[127]) fp8 | rnrm bf16]
    shard1 = nc.dram_tensor("shard1", [PPC, 128], dt.bfloat16)
    shard2 = nc.dram_tensor("shard2", [PPC, 128], dt.bfloat16)
    table1 = nc.dram_tensor("table1", [NTAB, 128], dt.bfloat16, addr_space="Shared")
    table2 = nc.dram_tensor("table2", [NTAB, 128], dt.bfloat16, addr_space="Shared")
    # private per-core copies: spread the random gather reads across each
    # core's own HBM instead of all 8 cores hammering the one shared buffer
    table1p = nc.dram_tensor("table1p", [NTAB, 128], dt.bfloat16)
    table2p = nc.dram_tensor("table2p", [NTAB, 128], dt.bfloat16)

    rg = [list(range(NC))]
    qctr = [0]

    with tile.TileContext(nc) as tc:
        with (
            tc.tile_pool(name="persist", bufs=1) as pp,
            tc.tile_pool(name="nodes", bufs=1) as np_pool,
            tc.tile_pool(name="gath", bufs=LOOKAHEAD + 1) as gp,
            tc.tile_pool(name="work", bufs=2) as wp,
            tc.tile_pool(name="xtile", bufs=2) as xtp,
            tc.tile_pool(name="small", bufs=6) as sp,
            tc.tile_pool(name="grp", bufs=2) as grp_pool,
            tc.tile_pool(name="psum", bufs=6, space="PSUM") as psp,
        ):
            idx_sb = pp.tile([128, meta.tot16], dt.int16)
            live_sb = pp.tile([128, meta.totC], dt.bfloat16)
            mvec_sb = pp.tile([128, NT], dt.float32)
            ident_sb = pp.tile([128, 128], dt.bfloat16)
            w1_sb = pp.tile([128, 3, HID], dt.bfloat16)
            beta2_sb = pp.tile([1, 1], dt.float32)
            beta2_col = pp.tile([128, 1], dt.float32)
            nc.sync.dma_start(idx_sb[:], idx_p[:])
            nc.sync.dma_start(live_sb[:], live_p[:])
            nc.sync.dma_start(mvec_sb[:], mvec_p[:])
            nc.sync.dma_start(ident_sb[:], ident_p[:])
            for k in range(3):
                nc.sync.dma_start(w1_sb[:, k, :], W1T3_p[k])
            nc.sync.dma_start(beta2_sb[:], beta2_p[:])
            nc.gpsimd.partition_broadcast(beta2_col[:], beta2_sb[:])

            h_nm = np_pool.tile([128, NT, 128], dt.bfloat16)
            rnrm_nm = np_pool.tile([128, NT], dt.float32)

            def rsqrt_batch(nrm2, G):
                # rnrm = rsqrt(max(nrm2, 1e-24)) via bit magic + 2 Newton
                # steps; nrm2 is a [128, G] f32 tile, modified in place.
                nc.vector.tensor_scalar_max(out=nrm2[:], in0=nrm2[:],
                                            scalar1=1e-24)
                rnrm = sp.tile([128, G], dt.float32, tag="rnrm")
                bits = rnrm[:].bitcast(dt.int32)
                nc.vector.tensor_scalar(
                    out=bits, in0=nrm2[:].bitcast(dt.int32), scalar1=1,
                    scalar2=None, op0=ALU.logical_shift_right,
                )
                nc.vector.tensor_scalar(
                    out=bits, in0=bits, scalar1=-1, scalar2=0x5F3759DF,
                    op0=ALU.mult, op1=ALU.add,
                )
                tmp = sp.tile([128, G], dt.float32, tag="rsq_t")
                for _ in range(2):
                    nc.vector.tensor_mul(out=tmp[:], in0=rnrm[:], in1=rnrm[:])
                    nc.vector.tensor_mul(out=tmp[:], in0=tmp[:], in1=nrm2[:])
                    nc.vector.tensor_scalar(
                        out=tmp[:], in0=tmp[:], scalar1=-0.5, scalar2=1.5,
                        op0=ALU.mult, op1=ALU.add,
                    )
                    nc.vector.tensor_mul(out=rnrm[:], in0=rnrm[:], in1=tmp[:])
                return rnrm

            def norm_and_store(ts, shard):
                # h_nm[:, t, :] already holds h (bf16) for t in ts; compute
                # rnrm, build the packed 256B rows, and write the shard.
                G = len(ts)
                t0, G0 = ts[0], len(ts)
                nrm2 = sp.tile([128, G], dt.float32, tag="nrm2")
                sq = wp.tile([128, G, 128], dt.bfloat16, tag="sqscratch")
                nc.vector.tensor_mul(out=sq[:], in0=h_nm[:, t0 : t0 + G, :],
                                     in1=h_nm[:, t0 : t0 + G, :])
                nc.vector.tensor_reduce(
                    out=nrm2[:], in_=sq[:], axis=AX.X, op=ALU.add,
                )
                rnrm = rsqrt_batch(nrm2, G)
                nc.vector.tensor_scalar(
                    out=rnrm_nm[:, t0 : t0 + G0], in0=rnrm[:],
                    scalar1=1.0, scalar2=None, op0=ALU.mult,
                )
                row = wp.tile([128, G, 128], dt.bfloat16, tag="rowt")
                nc.vector.tensor_scalar(
                    out=row[:, :, 0:126], in0=h_nm[:, t0 : t0 + G0, 0:126],
                    scalar1=1.0, scalar2=None, op0=ALU.mult,
                )
                nc.vector.tensor_scalar(
                    out=row[:, :, 126:127].bitcast(dt.float8e4),
                    in0=h_nm[:, t0 : t0 + G0, 126:128],
                    scalar1=1.0, scalar2=None, op0=ALU.mult,
                )
                nc.vector.tensor_scalar(
                    out=row[:, :, 127:128], in0=rnrm[:].unsqueeze(2),
                    scalar1=1.0, scalar2=None, op0=ALU.mult,
                )
                nc.sync.dma_start(
                    shard[t0 * 128 : (t0 + G0) * 128, 0:128]
                    .rearrange("(g p) f -> p g f", p=128),
                    row[:],
                )

            # chunked AllGather + private-copy staging: chunk k ships as
            # soon as the tiles covering its rows are written, overlapping
            # the collective and table copy with compute
            def make_stager(shard, table, tablep):
                state = [0]

                def stage(tiles_done):
                    while state[0] < 4 and tiles_done >= CHT[state[0]]:
                        k = state[0]
                        r0, r1 = CH[k], CH[k + 1]
                        o0, o1 = CHOFF[k], CHOFF[k + 1]
                        nc.gpsimd.collective_compute(
                            "AllGather", mybir.AluOpType.bypass,
                            ins=[shard[r0:r1, :]], outs=[table[o0:o1, :]],
                            replica_groups=rg,
                        )
                        nc.sync.dma_start(tablep[o0:o1, :], table[o0:o1, :])
                        state[0] += 1
                return stage

            stage1 = make_stager(shard1, table1, table1p)
            stage2 = make_stager(shard2, table2, table2p)

            # ---------------- phase 1: L1 + table1 ----------------
            def build_table1():
                for t0 in range(0, NT, l1_group):
                    ts = list(range(t0, min(t0 + l1_group, NT)))
                    G0 = len(ts)
                    xt_g = xtp.tile([128, 3, l1_group * 128], dt.bfloat16,
                                    tag="xt")
                    nc.sync.dma_start(
                        xt_g[:, :, 0 : G0 * 128],
                        xT3_p[:, :, t0 * 128 : (t0 + G0) * 128]
                        .transpose([1, 0, 2]),
                    )
                    for i, t in enumerate(ts):
                        ps = psp.tile([128, HID], dt.float32, tag="acc")
                        for k in range(3):
                            nc.tensor.matmul(
                                ps[:],
                                lhsT=xt_g[:, k, i * 128 : (i + 1) * 128],
                                rhs=w1_sb[:, k, :],
                                start=(k == 0),
                                stop=(k == 2),
                            )
                        nc.scalar.activation(h_nm[:, t, :], ps[:], ACTF.Relu)
                    norm_and_store(ts, shard1)
                    stage1(ts[-1] + 1)

            build_table1()

            # ---------------- conv phases ----------------
            def run_conv(table, beta_scale, beta_col_ap, writer):
                expb = sp.tile([128, 1], dt.float32, tag="expb")
                if beta_col_ap is None:
                    ones = sp.tile([128, 1], dt.float32, tag="ones1")
                    nc.vector.memset(ones[:], 1.0)
                    nc.scalar.activation(expb[:], ones[:], ACTF.Exp)
                else:
                    nc.scalar.activation(expb[:], beta_col_ap[:], ACTF.Exp)
                tabA = table[0:SPLIT, :]
                tabB = table[SPLIT:NTAB, :]

                groups = meta.groups
                ngrp = len(groups)
                # per-group gather layout (tiles padded to group-max counts)
                gsumA = [len(g) * meta.gCA[gi] for gi, g in enumerate(groups)]
                gsumB = [len(g) * meta.gCB[gi] for gi, g in enumerate(groups)]
                gtiles = {}
                i16offs = []
                i16off = 0
                for gi, g in enumerate(groups):
                    i16offs.append(i16off)
                    i16off += (gsumA[gi] + gsumB[gi]) * 8

                def issue_gather(gi):
                    g = groups[gi]
                    sumA, sumB = gsumA[gi], gsumB[gi]
                    gtA = gp.tile([128, max(sumA, 1), 128], dt.bfloat16,
                                  tag="gA")
                    gtB = gp.tile([128, max(sumB, 1), 128], dt.bfloat16,
                                  tag="gB")
                    gtiles[gi] = (gtA, gtB)
                    off16 = i16offs[gi]
                    for (cn, gt, tab) in ((sumA, gtA, tabA),
                                          (sumB, gtB, tabB)):
                        if cn == 0:
                            continue
                        # split each call across queues so the SWDGE drains
                        # of one group run in parallel
                        half = (cn + 1) // 2
                        for (c0, c1) in ((0, half), (half, cn)):
                            if c1 <= c0:
                                continue
                            nidx = (c1 - c0) * 128
                            n16 = nidx // 16
                            nc.gpsimd.dma_gather(
                                out_ap=gt[:, c0:c1, :],
                                in_ap=tab,
                                idxs_ap=idx_sb[:, off16 : off16 + n16],
                                num_idxs=nidx,
                                num_idxs_reg=nidx,
                                elem_size=128,
                                single_packet=False,
                                queue_num=qctr[0] % 4,
                            )
                            qctr[0] += 1
                            off16 += n16

                def conv_group(gi):
                    g = groups[gi]
                    gtA, gtB = gtiles.pop(gi)
                    G = len(g)
                    CAg, CBg = meta.gCA[gi], meta.gCB[gi]
                    SA, SB = G * CAg, G * CBg
                    S = SA + SB
                    goff = meta.goffs[gi]
                    t0 = g[0]

                    # self-loop scale per tile (batched over the group)
                    selfa = sp.tile([128, G], dt.float32, tag="selfa")
                    nc.vector.tensor_scalar(
                        out=selfa[:], in0=mvec_sb[:, t0 : t0 + G],
                        scalar1=expb[:], scalar2=None, op0=ALU.mult,
                    )
                    deng = sp.tile([128, G], dt.float32, tag="deng")

                    if S > 0:
                        prod = wp.tile([128, S, 128], dt.bfloat16, tag="prod")
                        cosg = sp.tile([128, S], dt.float32, tag="cosg")
                        for (cn, SN, po, gt) in ((CAg, SA, 0, gtA),
                                                 (CBg, SB, SA, gtB)):
                            if SN == 0:
                                continue
                            g4 = gt[:, 0:SN, 0:126].rearrange(
                                "p (g c) f -> p g c f", g=G)
                            nc.vector.tensor_tensor(
                                out=prod[:, po : po + SN, 0:126].rearrange(
                                    "p (g c) f -> p g c f", g=G),
                                in0=g4,
                                in1=h_nm[:, t0 : t0 + G, 0:126].unsqueeze(2)
                                .broadcast_to([128, G, cn, 126]),
                                op=ALU.mult,
                            )
                            g8 = gt[:, 0:SN, 126:127].bitcast(
                                dt.float8e4).rearrange(
                                "p (g c) f -> p g c f", g=G)
                            nc.vector.tensor_tensor(
                                out=prod[:, po : po + SN, 126:128].rearrange(
                                    "p (g c) f -> p g c f", g=G),
                                in0=g8,
                                in1=h_nm[:, t0 : t0 + G, 126:128].unsqueeze(2)
                                .broadcast_to([128, G, cn, 2]),
                                op=ALU.mult,
                            )
                            nc.vector.tensor_reduce(
                                out=cosg[:, po : po + SN],
                                in_=prod[:, po : po + SN, :],
                                axis=AX.X, op=ALU.add,
                            )
                            # raw dot -> cosine: * rnrm_src (gathered col
                            # 127) and * rnrm_dst (local, per tile)
                            nc.vector.tensor_tensor(
                                out=cosg[:, po : po + SN].unsqueeze(2),
                                in0=cosg[:, po : po + SN].unsqueeze(2),
                                in1=gt[:, 0:SN, 127:128],
                                op=ALU.mult,
                            )
                            nc.vector.tensor_tensor(
                                out=cosg[:, po : po + SN].rearrange(
                                    "p (g c) -> p g c", g=G),
                                in0=cosg[:, po : po + SN].rearrange(
                                    "p (g c) -> p g c", g=G),
                                in1=rnrm_nm[:, t0 : t0 + G].unsqueeze(2)
                                .broadcast_to([128, G, cn]),
                                op=ALU.mult,
                            )
                        alpha = sp.tile([128, S], dt.bfloat16, tag="alpha")
                        scale = beta_scale if beta_col_ap is None \
                            else beta_col_ap[:]
                        nc.scalar.activation(alpha[:], cosg[:], ACTF.Exp,
                                             scale=scale)
                        # denominator: alpha * live, reduced per tile
                        alpham = sp.tile([128, S], dt.float32, tag="alpham")
                        nc.vector.tensor_tensor(
                            out=alpham[:], in0=alpha[:],
                            in1=live_sb[:, goff : goff + S], op=ALU.mult,
                        )
                        den0 = sp.tile([128, 2 * G], dt.float32, tag="den0")
                        for (SN, po, do) in ((SA, 0, 0), (SB, SA, G)):
                            if SN:
                                nc.vector.tensor_reduce(
                                    out=den0[:, do : do + G],
                                    in_=alpham[:, po : po + SN].rearrange(
                                        "p (g c) -> p g c", g=G),
                                    axis=AX.X, op=ALU.add,
                                )
                            else:
                                nc.vector.memset(den0[:, do : do + G], 0.0)
                        dsum = sp.tile([128, G], dt.float32, tag="dsum")
                        nc.vector.tensor_tensor(
                            out=dsum[:], in0=den0[:, 0:G],
                            in1=den0[:, G : 2 * G], op=ALU.add,
                        )
                        nc.vector.scalar_tensor_tensor(
                            out=deng[:], in0=dsum[:], scalar=1e-16,
                            in1=selfa[:], op0=ALU.add, op1=ALU.add,
                        )
                        # scaled h rows (alpha broadcast along features)
                        for (cn, SN, po, gt) in ((CAg, SA, 0, gtA),
                                                 (CBg, SB, SA, gtB)):
                            if SN == 0:
                                continue
                            nc.vector.tensor_tensor(
                                out=prod[:, po : po + SN, 0:126],
                                in0=gt[:, 0:SN, 0:126],
                                in1=alpha[:, po : po + SN].unsqueeze(2)
                                .broadcast_to([128, SN, 126]),
                                op=ALU.mult,
                            )
                            nc.vector.tensor_tensor(
                                out=prod[:, po : po + SN, 126:128],
                                in0=gt[:, 0:SN, 126:127].bitcast(
                                    dt.float8e4),
                                in1=alpha[:, po : po + SN].unsqueeze(2)
                                .broadcast_to([128, SN, 2]),
                                op=ALU.mult,
                            )
                    else:
                        nc.vector.tensor_scalar(
                            out=deng[:], in0=selfa[:],
                            scalar1=1e-16, scalar2=None, op0=ALU.add,
                        )
                    # self chunks + per-tile matmul accumulation; drain each
                    # PSUM (unscaled) on ACT so nothing here waits on den
                    pself = wp.tile([128, G, 128], dt.bfloat16, tag="pself")
                    num = wp.tile([128, G, 128], dt.float32, tag="num")
                    nc.vector.tensor_tensor(
                        out=pself[:], in0=h_nm[:, t0 : t0 + G, :],
                        in1=selfa[:].unsqueeze(2).broadcast_to([128, G, 128]),
                        op=ALU.mult,
                    )
                    for i, t in enumerate(g):
                        ps = psp.tile([128, 128], dt.float32, tag="acc")
                        C = CAg + CBg
                        for cc in range(CAg):
                            nc.tensor.matmul(
                                ps[:], lhsT=ident_sb[:],
                                rhs=prod[:, i * CAg + cc, :],
                                start=(cc == 0), stop=False,
                            )
                        for cc in range(CBg):
                            nc.tensor.matmul(
                                ps[:], lhsT=ident_sb[:],
                                rhs=prod[:, SA + i * CBg + cc, :],
                                start=False, stop=False,
                            )
                        nc.tensor.matmul(
                            ps[:], lhsT=ident_sb[:], rhs=pself[:, i, :],
                            start=(C == 0), stop=True,
                        )
                        nc.scalar.activation(num[:, i, :], ps[:], ACTF.Copy)
                    state[gi] = (g, deng, num)

                def conv_phase2(gi):
                    g, deng, num = state.pop(gi)
                    G = len(g)
                    rden = sp.tile([128, G], dt.float32, tag="rdeng")
                    nc.vector.reciprocal(rden[:], deng[:])
                    writer(g, num, rden)

                state = {}
                for gi in range(ngrp + LOOKAHEAD + 1):
                    if gi < ngrp:
                        issue_gather(gi)
                    if LOOKAHEAD <= gi < ngrp + LOOKAHEAD:
                        conv_group(gi - LOOKAHEAD)
                    if gi > LOOKAHEAD:
                        conv_phase2(gi - LOOKAHEAD - 1)

            def conv1_writer(g, num, rden):
                G = len(g)
                nc.vector.tensor_tensor(
                    out=h_nm[:, g[0] : g[0] + G, :], in0=num[:],
                    in1=rden[:].unsqueeze(2).broadcast_to([128, G, 128]),
                    op=ALU.mult,
                )
                norm_and_store(list(g), shard2)
                stage2(g[-1] + 1)

            run_conv(table1p, 1.0, None, conv1_writer)

            def conv2_writer(g, num, rden):
                G = len(g)
                outg = grp_pool.tile([128, G, 128], dt.float32, tag="outg")
                nc.vector.tensor_tensor(
                    out=outg[:], in0=num[:],
                    in1=rden[:].unsqueeze(2).broadcast_to([128, G, 128]),
                    op=ALU.mult,
                )
                t0 = g[0]
                nc.sync.dma_start(
                    out_p[t0 * 128 : (t0 + G) * 128, :]
                    .rearrange("(g p) f -> p g f", p=128),
                    outg[:],
                )

            run_conv(table2p, None, beta2_col, conv2_writer)

    nc.compile()
    return nc


# ----------------------------------------------------------------------------
# Entry point
# ----------------------------------------------------------------------------
_CACHE = {}


def make_in_maps(x, W1, b1, beta2, meta):
    xT3, W1T3, ident = prep_inputs(x, W1, b1, meta)
    in_maps = []
    for c in range(NC):
        in_maps.append({
            "xT3": np.ascontiguousarray(xT3[c]),
            "W1T3": W1T3,
            "idx16": meta.idx16[c],
            "live": meta.live[c],
            "mvec": meta.mvec[c],
            "ident": ident,
            "beta2": np.asarray(beta2, np.float32).reshape(1, 1),
        })
    return in_maps


def assemble_out(results, meta):
    out = np.zeros((N_NODES, HID), np.float32)
    for c in range(NC):
        shard = results[c]["out"]
        nodes = meta.row2node[c * PPC : (c + 1) * PPC]
        valid = nodes >= 0
        out[nodes[valid]] = shard[valid]
    return out


def kernel(x, edge_index, W1, b1, beta2):
    _install_axon_prof_hook()
    from concourse.bass_utils import run_bass_kernel_spmd

    x = np.asarray(x, np.float32)
    edge_index = np.asarray(edge_index)
    W1 = np.asarray(W1, np.float32)
    b1 = np.asarray(b1, np.float32)
    beta2 = np.asarray(beta2, np.float32)

    key = (x.shape, edge_index.shape)
    if key not in _CACHE:
        meta = prep_graph(edge_index)
        nc = build_nc(meta)
        _CACHE[key] = (meta, nc, hash(edge_index.tobytes()))
    meta, nc, ehash = _CACHE[key]
    if ehash != hash(edge_index.tobytes()):
        meta = prep_graph(edge_index)
        nc = build_nc(meta)
        _CACHE[key] = (meta, nc, hash(edge_index.tobytes()))
        meta, nc, ehash = _CACHE[key]

    in_maps = make_in_maps(x, W1, b1, beta2, meta)
    res = run_bass_kernel_spmd(nc, in_maps, list(range(NC)))
    return assemble_out(res.results, meta)

